# revision 15
# baseline (speedup 1.0000x reference)
"""Shifted abs-diff cost volume kernel for Trainium2 (8 NeuronCores).

out[n, d, y, x] = |image1[n,0,y,x] - image2[n,0,y,x-d]|  (0 where x < d)

Sharding: pure data parallel over flattened (N*H) rows -> 96 rows/core.

Strategy: no abs / quantize compute on-chip at all. The host prescales
a' = S*a + 128, b' = S*b (fp16, S=14), so the DVE tensor_sub directly
produces the biased quantized value diff' = S*(a-b) + 128 in [8.6,
247.4] -- always positive, u8-range. Every output element then only
needs an fp16->u8 conversion: 4 of 8 disparity pair-blocks ride the ACT
engine (Abs = identity on positives, 1x rate, u8 out) + plain HWDGE
DMA, the other 4 go STRAIGHT from the fp16 diff tile to HBM via SWDGE
cast-DMA (fp16->u8 round-to-nearest in the DMA datapath). The host
dequant is |u8 - 128| / S, which also applies the abs. Total error
<= 0.66 u8 LSB = 0.047 abs (rel ~6e-3), inside the 2e-2 gate.

Layout is d-major per partition: diff tiles hold [i(16), slot(3),
x(312)] so a pair-block is one 14976-element contiguous run per
partition, and the DRAM output [128, D*SLOTS*SEG] matches -- every
output DMA descriptor moves 14976 contiguous bytes (vs 2.5 KB runs in
the row-major layout, which capped the DMA engines at ~55% line rate).

The DVE subtract chain (~65us busy) is the critical path; ACT (~52us)
and the DMA engines are hidden under it, and the drain pair is split
into i-halves feeding cast-DMAs so the tail flush is ~2 MB.

Per-core input layout (unchanged): rows split into 4 column quarters
of 312; the 96x4 quarter-segments pack onto 128 partitions (3
slots/partition). Each slot holds [img1 seg | img2 seg with
128-left-halo | same shifted by one]. The second img2 copy keeps TT
reads 4B-aligned for odd disparities (DVE 2x_1P mode); disparities run
in pair-blocks of 16 (8 even from copy E, 8 odd from copy O, AP stride
-2).

Queues: plain u8 + input DMAs on Sync (HWDGE), cast-DMAs on GpSimd
(SWDGE, the only engine allowed to cast).

The x<d wedge (zero by definition, data-independent) is filled by the
host during unshard, like the halo padding it mirrors.
"""

import numpy as np

import concourse.bass as bass
import concourse.tile as tile
from concourse import mybir
from concourse.ap import AP
from concourse.bass_utils import run_bass_kernel_spmd

N, C, H, W = 2, 1, 384, 1248
D = 128  # MAXDISP
NCORES = 8
ROWS = (N * H) // NCORES  # 96 rows per core
Q = 4  # column quarters per row
SEG = W // Q  # 312 columns per segment
SLOTS = ROWS * Q // 128  # 3 segments per partition
PADL = 128  # left zero pad of img2 (even copy); odd copy uses 127
REGION = SEG + PADL  # 440 columns per img2 copy
SLOT_COLS = SEG + 2 * REGION  # 1192: [img1 | img2 evenE | img2 oddO]
IN_COLS = SLOTS * SLOT_COLS  # 3576
GROUP = 8  # disparities per group
PAIR = 2 * GROUP  # 16 disparities per TT pair-block
NPAIRS = D // PAIR  # 8
IST = SLOTS * SEG  # 936: d-index stride inside a diff tile
PFREE = PAIR * IST  # 14976 free elems per pair diff tile
OUTROW = D * IST  # 119808 output cols per partition
ACT_PAIRS = (1, 3, 5)  # pairs converted fp16->u8 on ACT
QUARTER_PAIRS = (0, 7)  # ramp + drain: TT and cast-DMA in i-quarters
# cast pairs alternate with ACT pairs so both DMA queues are fed from
# ~13us on; the ramp pair casts quarters so output flows immediately
S = 14.0  # quant scale; |a-b| max ~8.53 -> diff' in [8.6, 247.4]
BIAS = 128.0
F16 = mybir.dt.float16
U8 = mybir.dt.uint8

_NC_CACHE = {}


def build_program():
    nc = bass.Bass("TRN2", target_bir_lowering=False, debug=False)
    imgs_d = nc.dram_tensor("images", [128, IN_COLS], F16, kind="ExternalInput").ap()
    # Per-core output, d-major per partition: [partition, d*SLOTS*SEG].
    out8_d = nc.dram_tensor("out8", [128, OUTROW], U8, kind="ExternalOutput").ap()

    with tile.TileContext(nc) as tc:
        with (
            tc.tile_pool(name="inp", bufs=1) as inp_pool,
            tc.tile_pool(name="diff", bufs=4) as diff_pool,
            tc.tile_pool(name="q8", bufs=3) as q8_pool,
        ):
            # Warm the ACT Abs table set off the critical path.
            warm = inp_pool.tile([128, 2], F16)
            nc.vector.memset(warm[:, :], 1.0)
            nc.scalar.activation(
                warm[:, :], warm[:, :], mybir.ActivationFunctionType.Abs
            )

            imgs = inp_pool.tile([128, IN_COLS], F16)
            nc.sync.dma_start(out=imgs[:, :], in_=imgs_d[:, :])

            def tt_pair(t, d0, i0=0, ni=PAIR, s=None):
                """diff[i, s, x] = img1[s,x] - img2[s, x-(d0+i)], i in
                [i0, i0+ni).

                Even i from copy E (base 440-d0), odd i from copy O (base
                878-d0); both strides -2 so every innermost run start stays
                4B-aligned -> DVE 2x_1P mode. i0/ni (i0 even) limit the
                disparity range (drain interleaving); s limits to one slot
                (ramp interleaving).
                """
                ng = ni // 2
                ns = SLOTS if s is None else 1
                sb = 0 if s is None else s
                for par, i1b in ((0, 440 - d0), (1, 878 - d0)):
                    out_ap = AP(
                        t.tensor,
                        (i0 + par) * IST + sb * SEG,
                        [[PFREE, 128], [2 * IST, ng], [SEG, ns], [1, SEG]],
                    )
                    in0 = AP(
                        imgs.tensor,
                        sb * SLOT_COLS,
                        [[IN_COLS, 128], [0, ng], [SLOT_COLS, ns], [1, SEG]],
                    )
                    in1 = AP(
                        imgs.tensor,
                        sb * SLOT_COLS + i1b - i0,
                        [[IN_COLS, 128], [-2, ng], [SLOT_COLS, ns], [1, SEG]],
                    )
                    nc.vector.tensor_sub(out_ap, in0, in1)

            def cast_dma(t, p, i0=0, ni=PAIR):
                """SWDGE cast-DMA: fp16 diff -> u8 straight to HBM."""
                nc.gpsimd.dma_start(
                    out=AP(
                        out8_d.tensor,
                        p * PFREE + i0 * IST,
                        [[OUTROW, 128], [1, ni * IST]],
                    ),
                    in_=AP(t.tensor, i0 * IST, [[PFREE, 128], [1, ni * IST]]),
                )

            def act_pair(t, p):
                """ACT fp16->u8 (Abs = identity on positives) + HWDGE DMA."""
                q = q8_pool.tile([128, PFREE], U8, tag="q8")
                nc.scalar.activation(
                    q[:, :],
                    AP(t.tensor, 0, [[PFREE, 128], [1, PFREE]]),
                    mybir.ActivationFunctionType.Abs,
                )
                nc.sync.dma_start(
                    out=AP(out8_d.tensor, p * PFREE, [[OUTROW, 128], [1, PFREE]]),
                    in_=q[:, :],
                )

            for p in range(NPAIRS):
                d0 = p * PAIR
                t = diff_pool.tile([128, PFREE], F16, tag="diff")
                if p in ACT_PAIRS:
                    tt_pair(t, d0)
                    act_pair(t, p)
                elif p in QUARTER_PAIRS:
                    # Ramp / drain: TT in i-quarters, each flushed by a
                    # cast-DMA while the next quarter's TTs run -- output
                    # flows ~13us in and the final flush is only ~0.94 MB.
                    for qi in range(4):
                        tt_pair(t, d0, i0=4 * qi, ni=4)
                        cast_dma(t, p, i0=4 * qi, ni=4)
                else:
                    # Cast pairs: TT in i-halves, each half flushed by a
                    # cast-DMA while the other half's TTs run.
                    for h in (0, 1):
                        tt_pair(t, d0, i0=GROUP * h, ni=GROUP)
                        cast_dma(t, p, i0=GROUP * h, ni=GROUP)
    return nc


def split_excess_waits(nc):
    """Split multi-wait instructions for this walrus build's ISA encoder.

    The TRN2 ISA encoding here holds 1 semaphore wait per engine
    instruction (2 for a standalone EventSemaphore). Tile's scheduler
    fuses up to ~3 waits per instruction, which this neuronxcc rejects
    with "Too many sync wait commands". Moving the excess waits into
    EventSemaphore instructions issued just before, on the same engine
    queue, is semantically identical (the engine stalls at the sync
    instruction instead).
    """
    counter = 0
    for f in nc.m.functions:
        for b in f.blocks:
            plan = []  # (index, [event_insts]) in original order
            insts = b.instructions
            for idx, inst in enumerate(insts):
                si = inst.sync_info
                if si is None:
                    continue
                waits = list(si.on_wait)
                cap = 2 if inst.opcode == "EventSemaphore" else 1
                if len(waits) <= cap:
                    continue
                extra, keep = waits[:-cap], waits[-cap:]
                evs = []
                for j in range(0, len(extra), 2):
                    ev = mybir.InstEventSemaphore(
                        name=f"EVWS-{counter}",
                        opcode="EventSemaphore",
                        engine=inst.engine,
                    )
                    counter += 1
                    ev.sync_info = mybir.SyncInfo(
                        on_wait=extra[j : j + 2], on_update=[]
                    )
                    evs.append(ev)
                inst.sync_info = mybir.SyncInfo(
                    on_wait=keep, on_update=list(si.on_update)
                )
                plan.append((idx, evs))
            # apply inserts back-to-front so earlier indices stay valid
            for idx, evs in reversed(plan):
                for k, ev in enumerate(evs):
                    insts.insert(idx + k, ev)
    return nc


def get_program():
    if "nc" not in _NC_CACHE:
        _NC_CACHE["nc"] = split_excess_waits(build_program())
    return _NC_CACHE["nc"]


def shard_inputs(image1, image2):
    img1 = np.asarray(image1, dtype=np.float32).reshape(N * H, W) * S + BIAS
    img2 = np.asarray(image2, dtype=np.float32).reshape(N * H, W) * S
    # 128-zero left pad (copy E); copy O reads the same shifted by one,
    # so pad one trailing zero too.
    img2p = np.concatenate(
        [np.zeros((N * H, PADL), np.float32), img2, np.zeros((N * H, 1), np.float32)],
        axis=1,
    )
    maps = []
    p = np.arange(128)
    c, rm = p // 32, p % 32
    xs = np.arange(SEG)
    xr = np.arange(REGION)
    for k in range(NCORES):
        i1 = img1[k * ROWS : (k + 1) * ROWS]
        i2 = img2p[k * ROWS : (k + 1) * ROWS]
        packed = np.empty((128, IN_COLS), np.float16)
        for s in range(SLOTS):
            r = 32 * s + rm
            base = s * SLOT_COLS
            packed[:, base : base + SEG] = i1[r[:, None], c[:, None] * SEG + xs]
            packed[:, base + SEG : base + SEG + REGION] = i2[
                r[:, None], c[:, None] * SEG + xr
            ]
            packed[:, base + SEG + REGION : base + SLOT_COLS] = i2[
                r[:, None], c[:, None] * SEG + 1 + xr
            ]
        maps.append({"images": np.ascontiguousarray(packed)})
    return maps


def unshard_output(results):
    out = np.empty((N, D * C, H, W), dtype=np.float32)
    for k in range(NCORES):
        # [partition(c,rm), d, slot, x] ; rows r = 32*slot + rm, col = c*SEG+x
        a8 = np.asarray(results[k]["out8"]).reshape(4, 32, D, SLOTS, SEG)
        full = np.abs(a8.astype(np.float32) - BIAS) * (1.0 / S)
        n = (k * ROWS) // H
        y0 = (k * ROWS) % H
        blk = full.transpose(2, 3, 1, 0, 4).reshape(D, ROWS, W)
        out[n, :, y0 : y0 + ROWS, :] = blk
    # x < d wedge is zero by definition (the shift window falls off the
    # left edge) -- data-independent padding, filled here like the halo.
    for d in range(1, D):
        out[:, d, :, :d] = 0.0
    return out


def kernel(image1, image2):
    nc = get_program()
    res = run_bass_kernel_spmd(nc, shard_inputs(image1, image2), list(range(NCORES)))
    return unshard_output(res.results)


# revision 16
# speedup vs baseline: 1.0321x; 1.0321x over previous
"""Shifted abs-diff cost volume kernel for Trainium2 (8 NeuronCores).

out[n, d, y, x] = |image1[n,0,y,x] - image2[n,0,y,x-d]|  (0 where x < d)

Sharding: pure data parallel over flattened (N*H) rows -> 96 rows/core.

Strategy: no abs / quantize compute on-chip at all. The host prescales
a' = S*a + 128, b' = S*b (fp16, S=14), so the DVE tensor_sub directly
produces the biased quantized value diff' = S*(a-b) + 128 in [8.6,
247.4] -- always positive, u8-range. Every output element then only
needs an fp16->u8 conversion: 4 of 8 disparity pair-blocks ride the ACT
engine (Abs = identity on positives, 1x rate, u8 out) + plain HWDGE
DMA, the other 4 go STRAIGHT from the fp16 diff tile to HBM via SWDGE
cast-DMA (fp16->u8 round-to-nearest in the DMA datapath). The host
dequant is |u8 - 128| / S, which also applies the abs. Total error
<= 0.66 u8 LSB = 0.047 abs (rel ~6e-3), inside the 2e-2 gate.

Layout is d-major per partition: diff tiles hold [i(16), slot(3),
x(312)] so a pair-block is one 14976-element contiguous run per
partition, and the DRAM output [128, D*SLOTS*SEG] matches -- every
output DMA descriptor moves 14976 contiguous bytes (vs 2.5 KB runs in
the row-major layout, which capped the DMA engines at ~55% line rate).

The DVE subtract chain (~65us busy) is the critical path; ACT (~52us)
and the DMA engines are hidden under it, and the drain pair is split
into i-halves feeding cast-DMAs so the tail flush is ~2 MB.

Per-core input layout (unchanged): rows split into 4 column quarters
of 312; the 96x4 quarter-segments pack onto 128 partitions (3
slots/partition). Each slot holds [img1 seg | img2 seg with
128-left-halo | same shifted by one]. The second img2 copy keeps TT
reads 4B-aligned for odd disparities (DVE 2x_1P mode); disparities run
in pair-blocks of 16 (8 even from copy E, 8 odd from copy O, AP stride
-2).

Queues: plain u8 + input DMAs on Sync (HWDGE), cast-DMAs on GpSimd
(SWDGE, the only engine allowed to cast).

The x<d wedge (zero by definition, data-independent) is filled by the
host during unshard, like the halo padding it mirrors.
"""

import numpy as np

import concourse.bass as bass
import concourse.tile as tile
from concourse import mybir
from concourse.ap import AP
from concourse.bass_utils import run_bass_kernel_spmd

N, C, H, W = 2, 1, 384, 1248
D = 128  # MAXDISP
NCORES = 8
ROWS = (N * H) // NCORES  # 96 rows per core
Q = 4  # column quarters per row
SEG = W // Q  # 312 columns per segment
SLOTS = ROWS * Q // 128  # 3 segments per partition
PADL = 128  # left zero pad of img2 (even copy); odd copy uses 127
REGION = SEG + PADL  # 440 columns per img2 copy
SLOT_COLS = SEG + 2 * REGION  # 1192: [img1 | img2 evenE | img2 oddO]
IN_COLS = SLOTS * SLOT_COLS  # 3576
GROUP = 8  # disparities per group
PAIR = 2 * GROUP  # 16 disparities per TT pair-block
NPAIRS = D // PAIR  # 8
IST = SLOTS * SEG  # 936: d-index stride inside a diff tile
PFREE = PAIR * IST  # 14976 free elems per pair diff tile
OUTROW = D * IST  # 119808 output cols per partition
# Every pair 0-6 sends its low i-half to ACT (fp16->u8, then plain
# HWDGE u8 DMA) and its high i-half straight to a SWDGE cast-DMA, so
# both DMA queues are fed every ~8us with no dead zones; pair 7 drains
# through cast quarters so the final flush is only ~0.94 MB.
S = 14.0  # quant scale; |a-b| max ~8.53 -> diff' in [8.6, 247.4]
BIAS = 128.0
F16 = mybir.dt.float16
U8 = mybir.dt.uint8

_NC_CACHE = {}


def build_program():
    nc = bass.Bass("TRN2", target_bir_lowering=False, debug=False)
    imgs_d = nc.dram_tensor("images", [128, IN_COLS], F16, kind="ExternalInput").ap()
    # Per-core output, d-major per partition: [partition, d*SLOTS*SEG].
    out8_d = nc.dram_tensor("out8", [128, OUTROW], U8, kind="ExternalOutput").ap()

    with tile.TileContext(nc) as tc:
        with (
            tc.tile_pool(name="inp", bufs=1) as inp_pool,
            tc.tile_pool(name="diff", bufs=4) as diff_pool,
            tc.tile_pool(name="q8", bufs=3) as q8_pool,
        ):
            # Warm the ACT Abs table set off the critical path.
            warm = inp_pool.tile([128, 2], F16)
            nc.vector.memset(warm[:, :], 1.0)
            nc.scalar.activation(
                warm[:, :], warm[:, :], mybir.ActivationFunctionType.Abs
            )

            # Input in two strided DMAs: the img1+evenE runs of every
            # slot first (all the first TT half's par-0 op needs), then
            # the oddO runs, so the DVE chain starts ~2us earlier.
            imgs = inp_pool.tile([128, IN_COLS], F16)
            nc.sync.dma_start(
                out=AP(imgs.tensor, 0, [[IN_COLS, 128], [SLOT_COLS, SLOTS], [1, 752]]),
                in_=AP(imgs_d.tensor, 0, [[IN_COLS, 128], [SLOT_COLS, SLOTS], [1, 752]]),
            )
            nc.sync.dma_start(
                out=AP(
                    imgs.tensor, 752, [[IN_COLS, 128], [SLOT_COLS, SLOTS], [1, REGION]]
                ),
                in_=AP(
                    imgs_d.tensor, 752, [[IN_COLS, 128], [SLOT_COLS, SLOTS], [1, REGION]]
                ),
            )

            def tt_pair(t, d0, i0=0, ni=PAIR, s=None):
                """diff[i, s, x] = img1[s,x] - img2[s, x-(d0+i)], i in
                [i0, i0+ni).

                Even i from copy E (base 440-d0), odd i from copy O (base
                878-d0); both strides -2 so every innermost run start stays
                4B-aligned -> DVE 2x_1P mode. i0/ni (i0 even) limit the
                disparity range (drain interleaving); s limits to one slot
                (ramp interleaving).
                """
                ng = ni // 2
                ns = SLOTS if s is None else 1
                sb = 0 if s is None else s
                for par, i1b in ((0, 440 - d0), (1, 878 - d0)):
                    out_ap = AP(
                        t.tensor,
                        (i0 + par) * IST + sb * SEG,
                        [[PFREE, 128], [2 * IST, ng], [SEG, ns], [1, SEG]],
                    )
                    in0 = AP(
                        imgs.tensor,
                        sb * SLOT_COLS,
                        [[IN_COLS, 128], [0, ng], [SLOT_COLS, ns], [1, SEG]],
                    )
                    in1 = AP(
                        imgs.tensor,
                        sb * SLOT_COLS + i1b - i0,
                        [[IN_COLS, 128], [-2, ng], [SLOT_COLS, ns], [1, SEG]],
                    )
                    nc.vector.tensor_sub(out_ap, in0, in1)

            def cast_dma(t, p, i0=0, ni=PAIR):
                """SWDGE cast-DMA: fp16 diff -> u8 straight to HBM."""
                nc.gpsimd.dma_start(
                    out=AP(
                        out8_d.tensor,
                        p * PFREE + i0 * IST,
                        [[OUTROW, 128], [1, ni * IST]],
                    ),
                    in_=AP(t.tensor, i0 * IST, [[PFREE, 128], [1, ni * IST]]),
                )

            def act_part(t, p, i0=0, ni=PAIR):
                """ACT fp16->u8 (Abs = identity on positives) + HWDGE DMA."""
                nf = ni * IST
                q = q8_pool.tile([128, nf], U8, tag="q8")
                nc.scalar.activation(
                    q[:, :],
                    AP(t.tensor, i0 * IST, [[PFREE, 128], [1, nf]]),
                    mybir.ActivationFunctionType.Abs,
                )
                nc.sync.dma_start(
                    out=AP(
                        out8_d.tensor,
                        p * PFREE + i0 * IST,
                        [[OUTROW, 128], [1, nf]],
                    ),
                    in_=q[:, :],
                )

            for p in range(NPAIRS):
                d0 = p * PAIR
                t = diff_pool.tile([128, PFREE], F16, tag="diff")
                if p < NPAIRS - 1:
                    tt_pair(t, d0, i0=0, ni=GROUP)
                    act_part(t, p, i0=0, ni=GROUP)
                    tt_pair(t, d0, i0=GROUP, ni=GROUP)
                    cast_dma(t, p, i0=GROUP, ni=GROUP)
                else:
                    # Drain: TT in i-quarters, each flushed by a cast-DMA
                    # while the next quarter's TTs run.
                    for qi in range(4):
                        tt_pair(t, d0, i0=4 * qi, ni=4)
                        cast_dma(t, p, i0=4 * qi, ni=4)
    return nc


def split_excess_waits(nc):
    """Split multi-wait instructions for this walrus build's ISA encoder.

    The TRN2 ISA encoding here holds 1 semaphore wait per engine
    instruction (2 for a standalone EventSemaphore). Tile's scheduler
    fuses up to ~3 waits per instruction, which this neuronxcc rejects
    with "Too many sync wait commands". Moving the excess waits into
    EventSemaphore instructions issued just before, on the same engine
    queue, is semantically identical (the engine stalls at the sync
    instruction instead).
    """
    counter = 0
    for f in nc.m.functions:
        for b in f.blocks:
            plan = []  # (index, [event_insts]) in original order
            insts = b.instructions
            for idx, inst in enumerate(insts):
                si = inst.sync_info
                if si is None:
                    continue
                waits = list(si.on_wait)
                cap = 2 if inst.opcode == "EventSemaphore" else 1
                if len(waits) <= cap:
                    continue
                extra, keep = waits[:-cap], waits[-cap:]
                evs = []
                for j in range(0, len(extra), 2):
                    ev = mybir.InstEventSemaphore(
                        name=f"EVWS-{counter}",
                        opcode="EventSemaphore",
                        engine=inst.engine,
                    )
                    counter += 1
                    ev.sync_info = mybir.SyncInfo(
                        on_wait=extra[j : j + 2], on_update=[]
                    )
                    evs.append(ev)
                inst.sync_info = mybir.SyncInfo(
                    on_wait=keep, on_update=list(si.on_update)
                )
                plan.append((idx, evs))
            # apply inserts back-to-front so earlier indices stay valid
            for idx, evs in reversed(plan):
                for k, ev in enumerate(evs):
                    insts.insert(idx + k, ev)
    return nc


def get_program():
    if "nc" not in _NC_CACHE:
        _NC_CACHE["nc"] = split_excess_waits(build_program())
    return _NC_CACHE["nc"]


def shard_inputs(image1, image2):
    img1 = np.asarray(image1, dtype=np.float32).reshape(N * H, W) * S + BIAS
    img2 = np.asarray(image2, dtype=np.float32).reshape(N * H, W) * S
    # 128-zero left pad (copy E); copy O reads the same shifted by one,
    # so pad one trailing zero too.
    img2p = np.concatenate(
        [np.zeros((N * H, PADL), np.float32), img2, np.zeros((N * H, 1), np.float32)],
        axis=1,
    )
    maps = []
    p = np.arange(128)
    c, rm = p // 32, p % 32
    xs = np.arange(SEG)
    xr = np.arange(REGION)
    for k in range(NCORES):
        i1 = img1[k * ROWS : (k + 1) * ROWS]
        i2 = img2p[k * ROWS : (k + 1) * ROWS]
        packed = np.empty((128, IN_COLS), np.float16)
        for s in range(SLOTS):
            r = 32 * s + rm
            base = s * SLOT_COLS
            packed[:, base : base + SEG] = i1[r[:, None], c[:, None] * SEG + xs]
            packed[:, base + SEG : base + SEG + REGION] = i2[
                r[:, None], c[:, None] * SEG + xr
            ]
            packed[:, base + SEG + REGION : base + SLOT_COLS] = i2[
                r[:, None], c[:, None] * SEG + 1 + xr
            ]
        maps.append({"images": np.ascontiguousarray(packed)})
    return maps


def unshard_output(results):
    out = np.empty((N, D * C, H, W), dtype=np.float32)
    for k in range(NCORES):
        # [partition(c,rm), d, slot, x] ; rows r = 32*slot + rm, col = c*SEG+x
        a8 = np.asarray(results[k]["out8"]).reshape(4, 32, D, SLOTS, SEG)
        full = np.abs(a8.astype(np.float32) - BIAS) * (1.0 / S)
        n = (k * ROWS) // H
        y0 = (k * ROWS) % H
        blk = full.transpose(2, 3, 1, 0, 4).reshape(D, ROWS, W)
        out[n, :, y0 : y0 + ROWS, :] = blk
    # x < d wedge is zero by definition (the shift window falls off the
    # left edge) -- data-independent padding, filled here like the halo.
    for d in range(1, D):
        out[:, d, :, :d] = 0.0
    return out


def kernel(image1, image2):
    nc = get_program()
    res = run_bass_kernel_spmd(nc, shard_inputs(image1, image2), list(range(NCORES)))
    return unshard_output(res.results)


# revision 18
# speedup vs baseline: 1.0467x; 1.0142x over previous
"""Shifted abs-diff cost volume kernel for Trainium2 (8 NeuronCores).

out[n, d, y, x] = |image1[n,0,y,x] - image2[n,0,y,x-d]|  (0 where x < d)

Sharding: pure data parallel over flattened (N*H) rows -> 96 rows/core.

Strategy: no abs / quantize compute on-chip at all. The host prescales
a' = S*a + 128, b' = S*b (fp16, S=14), so the DVE tensor_sub directly
produces the biased quantized value diff' = S*(a-b) + 128 in [8.6,
247.4] -- always positive, u8-range. Every output element then only
needs an fp16->u8 conversion: 4 of 8 disparity pair-blocks ride the ACT
engine (Abs = identity on positives, 1x rate, u8 out) + plain HWDGE
DMA, the other 4 go STRAIGHT from the fp16 diff tile to HBM via SWDGE
cast-DMA (fp16->u8 round-to-nearest in the DMA datapath). The host
dequant is |u8 - 128| / S, which also applies the abs. Total error
<= 0.66 u8 LSB = 0.047 abs (rel ~6e-3), inside the 2e-2 gate.

Layout is d-major per partition: diff tiles hold [i(16), slot(3),
x(312)] so a pair-block is one 14976-element contiguous run per
partition, and the DRAM output [128, D*SLOTS*SEG] matches -- every
output DMA descriptor moves 14976 contiguous bytes (vs 2.5 KB runs in
the row-major layout, which capped the DMA engines at ~55% line rate).

The DVE subtract chain (~65us busy) is the critical path; ACT (~52us)
and the DMA engines are hidden under it, and the drain pair is split
into i-halves feeding cast-DMAs so the tail flush is ~2 MB.

Per-core input layout (unchanged): rows split into 4 column quarters
of 312; the 96x4 quarter-segments pack onto 128 partitions (3
slots/partition). Each slot holds [img1 seg | img2 seg with
128-left-halo | same shifted by one]. The second img2 copy keeps TT
reads 4B-aligned for odd disparities (DVE 2x_1P mode); disparities run
in pair-blocks of 16 (8 even from copy E, 8 odd from copy O, AP stride
-2).

Queues: plain u8 + input DMAs on Sync (HWDGE), cast-DMAs on GpSimd
(SWDGE, the only engine allowed to cast).

The x<d wedge (zero by definition, data-independent) is filled by the
host during unshard, like the halo padding it mirrors.
"""

import numpy as np

import concourse.bass as bass
import concourse.tile as tile
from concourse import mybir
from concourse.ap import AP
from concourse.bass_utils import run_bass_kernel_spmd

N, C, H, W = 2, 1, 384, 1248
D = 128  # MAXDISP
NCORES = 8
ROWS = (N * H) // NCORES  # 96 rows per core
Q = 4  # column quarters per row
SEG = W // Q  # 312 columns per segment
SLOTS = ROWS * Q // 128  # 3 segments per partition
PADL = 128  # left zero pad of img2 (even copy); odd copy uses 127
REGION = SEG + PADL  # 440 columns per img2 copy
SLOT_COLS = SEG + 2 * REGION  # 1192: [img1 | img2 evenE | img2 oddO]
IN_COLS = SLOTS * SLOT_COLS  # 3576
GROUP = 8  # disparities per group
PAIR = 2 * GROUP  # 16 disparities per TT pair-block
NPAIRS = D // PAIR  # 8
IST = SLOTS * SEG  # 936: d-index stride inside a diff tile
PFREE = PAIR * IST  # 14976 free elems per pair diff tile
OUTROW = D * IST  # 119808 output cols per partition
# Every pair 0-6 sends its low i-half to ACT (fp16->u8, then plain
# HWDGE u8 DMA) and its high i-half straight to a SWDGE cast-DMA, so
# both DMA queues are fed every ~8us with no dead zones; pair 7 drains
# through cast quarters so the final flush is only ~0.94 MB.
S = 14.0  # quant scale; |a-b| max ~8.53 -> diff' in [8.6, 247.4]
BIAS = 128.0
F16 = mybir.dt.float16
U8 = mybir.dt.uint8

_NC_CACHE = {}


def build_program():
    nc = bass.Bass("TRN2", target_bir_lowering=False, debug=False)
    imgs_d = nc.dram_tensor("images", [128, IN_COLS], F16, kind="ExternalInput").ap()
    # Per-core output, d-major per partition: [partition, d*SLOTS*SEG].
    out8_d = nc.dram_tensor("out8", [128, OUTROW], U8, kind="ExternalOutput").ap()

    with tile.TileContext(nc) as tc:
        with (
            tc.tile_pool(name="inp", bufs=1) as inp_pool,
            tc.tile_pool(name="diff", bufs=4) as diff_pool,
            tc.tile_pool(name="q8", bufs=3) as q8_pool,
        ):
            # Warm the ACT Abs table set off the critical path.
            warm = inp_pool.tile([128, 2], F16)
            nc.vector.memset(warm[:, :], 1.0)
            nc.scalar.activation(
                warm[:, :], warm[:, :], mybir.ActivationFunctionType.Abs
            )

            imgs = inp_pool.tile([128, IN_COLS], F16)
            nc.sync.dma_start(out=imgs[:, :], in_=imgs_d[:, :])

            def tt_pair(t, d0, i0=0, ni=PAIR, s=None):
                """diff[i, s, x] = img1[s,x] - img2[s, x-(d0+i)], i in
                [i0, i0+ni).

                Even i from copy E (base 440-d0), odd i from copy O (base
                878-d0); both strides -2 so every innermost run start stays
                4B-aligned -> DVE 2x_1P mode. i0/ni (i0 even) limit the
                disparity range (drain interleaving); s limits to one slot
                (ramp interleaving).
                """
                ng = ni // 2
                ns = SLOTS if s is None else 1
                sb = 0 if s is None else s
                for par, i1b in ((0, 440 - d0), (1, 878 - d0)):
                    out_ap = AP(
                        t.tensor,
                        (i0 + par) * IST + sb * SEG,
                        [[PFREE, 128], [2 * IST, ng], [SEG, ns], [1, SEG]],
                    )
                    in0 = AP(
                        imgs.tensor,
                        sb * SLOT_COLS,
                        [[IN_COLS, 128], [0, ng], [SLOT_COLS, ns], [1, SEG]],
                    )
                    in1 = AP(
                        imgs.tensor,
                        sb * SLOT_COLS + i1b - i0,
                        [[IN_COLS, 128], [-2, ng], [SLOT_COLS, ns], [1, SEG]],
                    )
                    nc.vector.tensor_sub(out_ap, in0, in1)

            def cast_dma(t, p, i0=0, ni=PAIR):
                """SWDGE cast-DMA: fp16 diff -> u8 straight to HBM."""
                nc.gpsimd.dma_start(
                    out=AP(
                        out8_d.tensor,
                        p * PFREE + i0 * IST,
                        [[OUTROW, 128], [1, ni * IST]],
                    ),
                    in_=AP(t.tensor, i0 * IST, [[PFREE, 128], [1, ni * IST]]),
                )

            def act_part(t, p, i0=0, ni=PAIR):
                """ACT fp16->u8 (Abs = identity on positives) + HWDGE DMA."""
                nf = ni * IST
                q = q8_pool.tile([128, nf], U8, tag="q8")
                nc.scalar.activation(
                    q[:, :],
                    AP(t.tensor, i0 * IST, [[PFREE, 128], [1, nf]]),
                    mybir.ActivationFunctionType.Abs,
                )
                nc.sync.dma_start(
                    out=AP(
                        out8_d.tensor,
                        p * PFREE + i0 * IST,
                        [[OUTROW, 128], [1, nf]],
                    ),
                    in_=q[:, :],
                )

            for p in range(NPAIRS):
                d0 = p * PAIR
                t = diff_pool.tile([128, PFREE], F16, tag="diff")
                if p < NPAIRS - 1:
                    tt_pair(t, d0, i0=0, ni=GROUP)
                    act_part(t, p, i0=0, ni=GROUP)
                    tt_pair(t, d0, i0=GROUP, ni=GROUP)
                    if p == NPAIRS - 2:
                        # ACT is idle by now; converting this half on ACT
                        # keeps the late SWDGE queue down to just the drain
                        # quarters, cutting the post-TT flush tail.
                        act_part(t, p, i0=GROUP, ni=GROUP)
                    else:
                        cast_dma(t, p, i0=GROUP, ni=GROUP)
                else:
                    # Drain: TT in i-quarters, each flushed by a cast-DMA
                    # while the next quarter's TTs run.
                    for qi in range(4):
                        tt_pair(t, d0, i0=4 * qi, ni=4)
                        cast_dma(t, p, i0=4 * qi, ni=4)
    return nc


def split_excess_waits(nc):
    """Split multi-wait instructions for this walrus build's ISA encoder.

    The TRN2 ISA encoding here holds 1 semaphore wait per engine
    instruction (2 for a standalone EventSemaphore). Tile's scheduler
    fuses up to ~3 waits per instruction, which this neuronxcc rejects
    with "Too many sync wait commands". Moving the excess waits into
    EventSemaphore instructions issued just before, on the same engine
    queue, is semantically identical (the engine stalls at the sync
    instruction instead).
    """
    counter = 0
    for f in nc.m.functions:
        for b in f.blocks:
            plan = []  # (index, [event_insts]) in original order
            insts = b.instructions
            for idx, inst in enumerate(insts):
                si = inst.sync_info
                if si is None:
                    continue
                waits = list(si.on_wait)
                cap = 2 if inst.opcode == "EventSemaphore" else 1
                if len(waits) <= cap:
                    continue
                extra, keep = waits[:-cap], waits[-cap:]
                evs = []
                for j in range(0, len(extra), 2):
                    ev = mybir.InstEventSemaphore(
                        name=f"EVWS-{counter}",
                        opcode="EventSemaphore",
                        engine=inst.engine,
                    )
                    counter += 1
                    ev.sync_info = mybir.SyncInfo(
                        on_wait=extra[j : j + 2], on_update=[]
                    )
                    evs.append(ev)
                inst.sync_info = mybir.SyncInfo(
                    on_wait=keep, on_update=list(si.on_update)
                )
                plan.append((idx, evs))
            # apply inserts back-to-front so earlier indices stay valid
            for idx, evs in reversed(plan):
                for k, ev in enumerate(evs):
                    insts.insert(idx + k, ev)
    return nc


def get_program():
    if "nc" not in _NC_CACHE:
        _NC_CACHE["nc"] = split_excess_waits(build_program())
    return _NC_CACHE["nc"]


def shard_inputs(image1, image2):
    img1 = np.asarray(image1, dtype=np.float32).reshape(N * H, W) * S + BIAS
    img2 = np.asarray(image2, dtype=np.float32).reshape(N * H, W) * S
    # 128-zero left pad (copy E); copy O reads the same shifted by one,
    # so pad one trailing zero too.
    img2p = np.concatenate(
        [np.zeros((N * H, PADL), np.float32), img2, np.zeros((N * H, 1), np.float32)],
        axis=1,
    )
    maps = []
    p = np.arange(128)
    c, rm = p // 32, p % 32
    xs = np.arange(SEG)
    xr = np.arange(REGION)
    for k in range(NCORES):
        i1 = img1[k * ROWS : (k + 1) * ROWS]
        i2 = img2p[k * ROWS : (k + 1) * ROWS]
        packed = np.empty((128, IN_COLS), np.float16)
        for s in range(SLOTS):
            r = 32 * s + rm
            base = s * SLOT_COLS
            packed[:, base : base + SEG] = i1[r[:, None], c[:, None] * SEG + xs]
            packed[:, base + SEG : base + SEG + REGION] = i2[
                r[:, None], c[:, None] * SEG + xr
            ]
            packed[:, base + SEG + REGION : base + SLOT_COLS] = i2[
                r[:, None], c[:, None] * SEG + 1 + xr
            ]
        maps.append({"images": np.ascontiguousarray(packed)})
    return maps


def unshard_output(results):
    out = np.empty((N, D * C, H, W), dtype=np.float32)
    for k in range(NCORES):
        # [partition(c,rm), d, slot, x] ; rows r = 32*slot + rm, col = c*SEG+x
        a8 = np.asarray(results[k]["out8"]).reshape(4, 32, D, SLOTS, SEG)
        full = np.abs(a8.astype(np.float32) - BIAS) * (1.0 / S)
        n = (k * ROWS) // H
        y0 = (k * ROWS) % H
        blk = full.transpose(2, 3, 1, 0, 4).reshape(D, ROWS, W)
        out[n, :, y0 : y0 + ROWS, :] = blk
    # x < d wedge is zero by definition (the shift window falls off the
    # left edge) -- data-independent padding, filled here like the halo.
    for d in range(1, D):
        out[:, d, :, :d] = 0.0
    return out


def kernel(image1, image2):
    nc = get_program()
    res = run_bass_kernel_spmd(nc, shard_inputs(image1, image2), list(range(NCORES)))
    return unshard_output(res.results)


# revision 19
# speedup vs baseline: 1.0707x; 1.0229x over previous
"""Shifted abs-diff cost volume kernel for Trainium2 (8 NeuronCores).

out[n, d, y, x] = |image1[n,0,y,x] - image2[n,0,y,x-d]|  (0 where x < d)

Sharding: pure data parallel over flattened (N*H) rows -> 96 rows/core.

Strategy: no abs / quantize compute on-chip at all. The host prescales
a' = S*a + 128, b' = S*b (fp16, S=14), so the DVE tensor_sub directly
produces the biased quantized value diff' = S*(a-b) + 128 in [8.6,
247.4] -- always positive, u8-range. Every output element then only
needs an fp16->u8 conversion: 4 of 8 disparity pair-blocks ride the ACT
engine (Abs = identity on positives, 1x rate, u8 out) + plain HWDGE
DMA, the other 4 go STRAIGHT from the fp16 diff tile to HBM via SWDGE
cast-DMA (fp16->u8 round-to-nearest in the DMA datapath). The host
dequant is |u8 - 128| / S, which also applies the abs. Total error
<= 0.66 u8 LSB = 0.047 abs (rel ~6e-3), inside the 2e-2 gate.

Layout is d-major per partition: diff tiles hold [i(16), slot(3),
x(312)] so a pair-block is one 14976-element contiguous run per
partition, and the DRAM output [128, D*SLOTS*SEG] matches -- every
output DMA descriptor moves 14976 contiguous bytes (vs 2.5 KB runs in
the row-major layout, which capped the DMA engines at ~55% line rate).

The DVE subtract chain (~65us busy) is the critical path; ACT (~52us)
and the DMA engines are hidden under it, and the drain pair is split
into i-halves feeding cast-DMAs so the tail flush is ~2 MB.

Per-core input layout (unchanged): rows split into 4 column quarters
of 312; the 96x4 quarter-segments pack onto 128 partitions (3
slots/partition). Each slot holds [img1 seg | img2 seg with
128-left-halo | same shifted by one]. The second img2 copy keeps TT
reads 4B-aligned for odd disparities (DVE 2x_1P mode); disparities run
in pair-blocks of 16 (8 even from copy E, 8 odd from copy O, AP stride
-2).

Queues: plain u8 + input DMAs on Sync (HWDGE), cast-DMAs on GpSimd
(SWDGE, the only engine allowed to cast).

The x<d wedge (zero by definition, data-independent) is filled by the
host during unshard, like the halo padding it mirrors.
"""

import numpy as np

import concourse.bass as bass
import concourse.tile as tile
from concourse import mybir
from concourse.ap import AP
from concourse.bass_utils import run_bass_kernel_spmd

N, C, H, W = 2, 1, 384, 1248
D = 128  # MAXDISP
NCORES = 8
ROWS = (N * H) // NCORES  # 96 rows per core
Q = 4  # column quarters per row
SEG = W // Q  # 312 columns per segment
SLOTS = ROWS * Q // 128  # 3 segments per partition
PADL = 128  # left zero pad of img2 (even copy); odd copy uses 127
REGION = SEG + PADL  # 440 columns per img2 copy
SLOT_COLS = SEG + 2 * REGION  # 1192: [img1 | img2 evenE | img2 oddO]
IN_COLS = SLOTS * SLOT_COLS  # 3576
GROUP = 8  # disparities per group
PAIR = 2 * GROUP  # 16 disparities per TT pair-block
NPAIRS = D // PAIR  # 8
IST = SLOTS * SEG  # 936: d-index stride inside a diff tile
PFREE = PAIR * IST  # 14976 free elems per pair diff tile
HALF = GROUP * IST  # 7488: one parity's block inside a pair tile
OUTROW = D * IST  # 119808 output cols per partition
EPART = SEG + REGION  # 752: per-slot [img1 | evenE] input block
OBASE = SLOTS * EPART  # 2256: start of the odd-copy input blocks
# Diff tiles are parity-major: the even-disparity TT op writes
# [0:HALF] and the odd one [HALF:2*HALF], so each pair-wide TT op is
# itself a consumer granule. Every pair 0-6 sends its even half to ACT
# (fp16->u8 + plain HWDGE u8 DMA) and its odd half straight to a SWDGE
# cast-DMA, so both DMA queues are fed every ~8us with no dead zones;
# pair 6's odd half also rides ACT (idle by then) and pair 7 drains
# through four cast quarters so the final flush is only ~0.94 MB.
S = 14.0  # quant scale; |a-b| max ~8.53 -> diff' in [8.6, 247.4]
BIAS = 128.0
F16 = mybir.dt.float16
U8 = mybir.dt.uint8

_NC_CACHE = {}


def build_program():
    nc = bass.Bass("TRN2", target_bir_lowering=False, debug=False)
    imgs_d = nc.dram_tensor("images", [128, IN_COLS], F16, kind="ExternalInput").ap()
    # Per-core output, d-major per partition: [partition, d*SLOTS*SEG].
    out8_d = nc.dram_tensor("out8", [128, OUTROW], U8, kind="ExternalOutput").ap()

    with tile.TileContext(nc) as tc:
        with (
            tc.tile_pool(name="inp", bufs=1) as inp_pool,
            tc.tile_pool(name="diff", bufs=4) as diff_pool,
            tc.tile_pool(name="q8", bufs=3) as q8_pool,
        ):
            # Warm the ACT Abs table set off the critical path.
            warm = inp_pool.tile([128, 2], F16)
            nc.vector.memset(warm[:, :], 1.0)
            nc.scalar.activation(
                warm[:, :], warm[:, :], mybir.ActivationFunctionType.Abs
            )

            # Input layout [img1|evenE per slot ... | oddO per slot]:
            # the first DMA carries everything the even-parity TT ops
            # read, so the DVE chain starts ~2.5us earlier.
            imgs = inp_pool.tile([128, IN_COLS], F16)
            nc.sync.dma_start(out=imgs[:, :OBASE], in_=imgs_d[:, :OBASE])
            nc.sync.dma_start(out=imgs[:, OBASE:], in_=imgs_d[:, OBASE:])

            def tt_par(t, d0, par, j0=0, ng=GROUP):
                """diff for disparities d = d0 + 2j + par, j in [j0, j0+ng).

                par=0 reads copy E, par=1 copy O (pre-shifted by one), so
                every innermost run start stays 4B-aligned -> DVE 2x_1P
                mode. Output block [par*HALF + j*IST : ...] is parity-major
                contiguous.
                """
                if par == 0:
                    i1b, sst = 440 - d0, EPART
                else:
                    i1b, sst = OBASE + 126 - d0, REGION
                out_ap = AP(
                    t.tensor,
                    par * HALF + j0 * IST,
                    [[PFREE, 128], [IST, ng], [SEG, SLOTS], [1, SEG]],
                )
                in0 = AP(
                    imgs.tensor,
                    0,
                    [[IN_COLS, 128], [0, ng], [EPART, SLOTS], [1, SEG]],
                )
                in1 = AP(
                    imgs.tensor,
                    i1b - 2 * j0,
                    [[IN_COLS, 128], [-2, ng], [sst, SLOTS], [1, SEG]],
                )
                nc.vector.tensor_sub(out_ap, in0, in1)

            def cast_dma(t, p, e0, nf):
                """SWDGE cast-DMA: fp16 diff -> u8 straight to HBM."""
                nc.gpsimd.dma_start(
                    out=AP(
                        out8_d.tensor,
                        p * PFREE + e0,
                        [[OUTROW, 128], [1, nf]],
                    ),
                    in_=AP(t.tensor, e0, [[PFREE, 128], [1, nf]]),
                )

            def act_part(t, p, e0, nf):
                """ACT fp16->u8 (Abs = identity on positives) + HWDGE DMA."""
                q = q8_pool.tile([128, nf], U8, tag="q8")
                nc.scalar.activation(
                    q[:, :],
                    AP(t.tensor, e0, [[PFREE, 128], [1, nf]]),
                    mybir.ActivationFunctionType.Abs,
                )
                nc.sync.dma_start(
                    out=AP(
                        out8_d.tensor,
                        p * PFREE + e0,
                        [[OUTROW, 128], [1, nf]],
                    ),
                    in_=q[:, :],
                )

            for p in range(NPAIRS):
                d0 = p * PAIR
                t = diff_pool.tile([128, PFREE], F16, tag="diff")
                if p < NPAIRS - 1:
                    tt_par(t, d0, 0)
                    act_part(t, p, 0, HALF)
                    tt_par(t, d0, 1)
                    if p == NPAIRS - 2:
                        # ACT is idle by now; converting this half on ACT
                        # keeps the late SWDGE queue down to just the drain
                        # quarters, cutting the post-TT flush tail.
                        act_part(t, p, HALF, HALF)
                    else:
                        cast_dma(t, p, HALF, HALF)
                else:
                    # Drain: TT in parity-quarters, each flushed by a
                    # cast-DMA while the next quarter's TTs run.
                    for par in (0, 1):
                        for jh in (0, 1):
                            tt_par(t, d0, par, j0=4 * jh, ng=4)
                            cast_dma(t, p, par * HALF + 4 * jh * IST, 4 * IST)
    return nc


def split_excess_waits(nc):
    """Split multi-wait instructions for this walrus build's ISA encoder.

    The TRN2 ISA encoding here holds 1 semaphore wait per engine
    instruction (2 for a standalone EventSemaphore). Tile's scheduler
    fuses up to ~3 waits per instruction, which this neuronxcc rejects
    with "Too many sync wait commands". Moving the excess waits into
    EventSemaphore instructions issued just before, on the same engine
    queue, is semantically identical (the engine stalls at the sync
    instruction instead).
    """
    counter = 0
    for f in nc.m.functions:
        for b in f.blocks:
            plan = []  # (index, [event_insts]) in original order
            insts = b.instructions
            for idx, inst in enumerate(insts):
                si = inst.sync_info
                if si is None:
                    continue
                waits = list(si.on_wait)
                cap = 2 if inst.opcode == "EventSemaphore" else 1
                if len(waits) <= cap:
                    continue
                extra, keep = waits[:-cap], waits[-cap:]
                evs = []
                for j in range(0, len(extra), 2):
                    ev = mybir.InstEventSemaphore(
                        name=f"EVWS-{counter}",
                        opcode="EventSemaphore",
                        engine=inst.engine,
                    )
                    counter += 1
                    ev.sync_info = mybir.SyncInfo(
                        on_wait=extra[j : j + 2], on_update=[]
                    )
                    evs.append(ev)
                inst.sync_info = mybir.SyncInfo(
                    on_wait=keep, on_update=list(si.on_update)
                )
                plan.append((idx, evs))
            # apply inserts back-to-front so earlier indices stay valid
            for idx, evs in reversed(plan):
                for k, ev in enumerate(evs):
                    insts.insert(idx + k, ev)
    return nc


def get_program():
    if "nc" not in _NC_CACHE:
        _NC_CACHE["nc"] = split_excess_waits(build_program())
    return _NC_CACHE["nc"]


def shard_inputs(image1, image2):
    img1 = np.asarray(image1, dtype=np.float32).reshape(N * H, W) * S + BIAS
    img2 = np.asarray(image2, dtype=np.float32).reshape(N * H, W) * S
    # 128-zero left pad (copy E); copy O reads the same shifted by one,
    # so pad one trailing zero too.
    img2p = np.concatenate(
        [np.zeros((N * H, PADL), np.float32), img2, np.zeros((N * H, 1), np.float32)],
        axis=1,
    )
    maps = []
    p = np.arange(128)
    c, rm = p // 32, p % 32
    xs = np.arange(SEG)
    xr = np.arange(REGION)
    for k in range(NCORES):
        i1 = img1[k * ROWS : (k + 1) * ROWS]
        i2 = img2p[k * ROWS : (k + 1) * ROWS]
        packed = np.empty((128, IN_COLS), np.float16)
        for s in range(SLOTS):
            r = 32 * s + rm
            base = s * EPART
            packed[:, base : base + SEG] = i1[r[:, None], c[:, None] * SEG + xs]
            packed[:, base + SEG : base + EPART] = i2[
                r[:, None], c[:, None] * SEG + xr
            ]
            ob = OBASE + s * REGION
            packed[:, ob : ob + REGION] = i2[
                r[:, None], c[:, None] * SEG + 1 + xr
            ]
        maps.append({"images": np.ascontiguousarray(packed)})
    return maps


def unshard_output(results):
    out = np.empty((N, D * C, H, W), dtype=np.float32)
    for k in range(NCORES):
        # [partition(c,rm), pair, parity, j, slot, x]; d = 16*pair+2j+par
        a8 = np.asarray(results[k]["out8"]).reshape(
            4, 32, NPAIRS, 2, GROUP, SLOTS, SEG
        )
        full = np.abs(a8.astype(np.float32) - BIAS) * (1.0 / S)
        n = (k * ROWS) // H
        y0 = (k * ROWS) % H
        # -> [pair, j, parity, slot, rm, c, x] so (pair, j, parity) flattens
        # to the d axis in d = 16*pair + 2*j + parity order
        blk = full.transpose(2, 4, 3, 5, 1, 0, 6).reshape(D, ROWS, W)
        out[n, :, y0 : y0 + ROWS, :] = blk
    # x < d wedge is zero by definition (the shift window falls off the
    # left edge) -- data-independent padding, filled here like the halo.
    for d in range(1, D):
        out[:, d, :, :d] = 0.0
    return out


def kernel(image1, image2):
    nc = get_program()
    res = run_bass_kernel_spmd(nc, shard_inputs(image1, image2), list(range(NCORES)))
    return unshard_output(res.results)
